# revision 97
# baseline (speedup 1.0000x reference)
"""Trainium2 Bass kernel for ContextualAttentionModule.

Data-parallel over batch: 8 samples -> 8 NeuronCores, one sample per core.
Per-core pipeline (C=256, H=W=32, L=1024 patches):
  scores  = <fg_patch(p), bg_patch(l)> via fp8e4 DoubleRow matmuls, hi-term
            only (the recovered/attention branch carries ~3% of output
            energy, so fp8 rounding there is far under the error budget)
  norm    = rsqrt(3x3-boxsum(colsum bg_masked^2) + 2304*eps^2), bf16 boxsum
  prop    = 3x3 window-sum of scores + eps*boxbox(colsum fg)  (DVE, bf16)
  E       = exp(prop * rn) in bf16 (no max-subtract)
  D, s2   = interleaved per-block [1,512] PE accumulations over E (lag 2)
  T8      = fp8(E * (1/D) * min(rn*2048, 256))  -- the 2^11 gain keeps the
            attn operand in fp8's normal range; undone exactly via the
            mask/(9*2048) constant tile.  1/D read directly from PSUM.
  recov   = fp8 DoubleRow tconv: t-block pairs contract in one matmul
            (lhs = transposed masked-bg banks quantized to fp8, rhs = T8)
  final   = (recov*mask/(9*2048)*2048 + eps*box(s2/D))*... + fg*(1-mask)
  out     = concat_g relu(dilated_conv_r(final) + b)  bf16 weights
"""

import numpy as np
import ml_dtypes

import concourse.bass as bass
import concourse.tile as tile
from concourse import bacc, mybir
from concourse.bass_utils import run_bass_kernel_spmd
from concourse.masks import make_identity

F32 = mybir.dt.float32
F32R = mybir.dt.float32r
BF16 = mybir.dt.bfloat16
F8 = mybir.dt.float8e4
U8 = mybir.dt.uint8
U16 = mybir.dt.uint16
AF = mybir.ActivationFunctionType
ALU = mybir.AluOpType
DR = mybir.MatmulPerfMode.DoubleRow

EPS = 1e-7
TS = 2048.0          # power-of-2 gain on the attn operand for fp8 range
RATES = (1, 2, 4, 8)
OFFS = [(dy, dx) for dy in range(3) for dx in range(3)]
# dy=1 taps first so the start=True matmul covers the full psum tile
TAPS = [(1, 0), (1, 1), (1, 2), (0, 0), (0, 1), (0, 2), (2, 0), (2, 1), (2, 2)]

_CACHE = {}


def _ring_zero(nc, buf, n=34, eng=None):
    """Zero only the 1-wide border ring of a [P, n, n] padded buffer."""
    eng = eng or nc.vector
    bc = U16 if buf.dtype == BF16 else F32
    eng.memset(buf[:, 0:n:n - 1, :].bitcast(bc), 0)
    eng.memset(buf[:, 1:n - 1, 0:n:n - 1].bitcast(bc), 0)


def _ring_zero_pair(nc, buf, eng=None):
    """Ring-zero both j-halves of a [P, 2, 34, 34] paired buffer."""
    eng = eng or nc.vector
    bc = U16 if buf.dtype == BF16 else U8
    eng.memset(buf[:, :, 0:34:33, :].bitcast(bc), 0)
    eng.memset(buf[:, :, 1:33, 0:34:33].bitcast(bc), 0)


def _boxsum(nc, scr, src_pad, dst_flat, eng=None):
    """3x3 SAME window sum: [1,34,34] ring-zero padded -> [1,32,32] flat."""
    eng = eng or nc.vector
    eng.tensor_tensor(scr[:, 1:33, 1:33], src_pad[:, 1:33, 0:32],
                      src_pad[:, 1:33, 1:33], ALU.add)
    eng.tensor_tensor(scr[:, 1:33, 1:33], scr[:, 1:33, 1:33],
                      src_pad[:, 1:33, 2:34], ALU.add)
    eng.tensor_tensor(dst_flat[:], scr[:, 0:32, 1:33],
                      scr[:, 1:33, 1:33], ALU.add)
    eng.tensor_tensor(dst_flat[:], dst_flat[:], scr[:, 2:34, 1:33], ALU.add)


def build_program():
    nc = bacc.Bacc()
    fg_d = nc.declare_dram_parameter("fg", [2, 128, 32, 32], BF16, isOutput=False)
    fg8_d = nc.declare_dram_parameter("fg8", [128, 3, 2, 32, 32], F8,
                                      isOutput=False)
    bgb_d = nc.declare_dram_parameter("bgm", [128, 2, 32, 32], BF16,
                                      isOutput=False)
    bg8_d = nc.declare_dram_parameter("bg8", [128, 3, 2, 32, 32], F8,
                                      isOutput=False)
    mask_d = nc.declare_dram_parameter("maskrow", [1, 1024], F32, isOutput=False)
    w8h_d = nc.declare_dram_parameter("w8h", [128, 2, 2304], F8, isOutput=False)
    w8l_d = nc.declare_dram_parameter("w8l", [128, 2, 2304], F8, isOutput=False)
    b_d = nc.declare_dram_parameter("bias", [64, 4], F32, isOutput=False)
    out_d = nc.declare_dram_parameter("out", [256, 32, 32], F32, isOutput=True)

    with tile.TileContext(nc) as tc:
        _emit(nc, tc, fg_d, fg8_d, bgb_d, bg8_d, mask_d, w8h_d, w8l_d,
              b_d, out_d)
    nc.compile()
    return nc


def _emit(nc, tc, fg_d, fg8_d, bgb_d, bg8_d, mask_d, w8h_d, w8l_d,
          b_d, out_d):
    with tc.tile_pool(name="main", bufs=1) as main:
        # ---------------- long-lived tiles ----------------
        fflat = main.tile([128, 2, 32, 32], BF16, name="fflat")
        invmaskb = main.tile([128, 32, 32], F32, name="invmaskb")
        maskb9 = main.tile([128, 32, 32], F32, name="maskb9")
        maskb9s = main.tile([128, 32, 32], F32, name="maskb9s")
        ones_col = main.tile([128, 1], F32R, name="ones_col")
        idR = main.tile([128, 128], BF16, name="idR")
        rncol = main.tile([128, 8], F32, name="rncol")
        rnccol = main.tile([128, 8], F32, name="rnccol")
        ercol = main.tile([128, 8], BF16, name="ercol")
        onesB = main.tile([128, 1], BF16, name="onesB")
        Gb = main.tile([128, 32, 32], BF16, name="Gb")
        msrow = main.tile([1, 1024], F32, name="msrow")
        w8h = main.tile([128, 2, 2304], F8, name="w8h")
        w8l = main.tile([128, 2, 2304], F8, name="w8l")
        f8h = main.tile([128, 2, 48, 48], F8, name="f8h")
        f8l = main.tile([128, 2, 48, 48], F8, name="f8l")
        biasb = main.tile([64, 4], F32, name="biasb")
        Ap = [main.tile([128, 2, 34, 34], BF16, name=f"Ap{q}") for q in range(4)]
        A = [Ap[t // 2][:, t % 2] for t in range(8)]
        T8p = [main.tile([128, 2, 34, 34], F8, name=f"T8p{q}") for q in range(4)]
        W = [main.tile([128, 34, 34], BF16, name=f"W{i}") for i in range(2)]

        with tc.tile_pool(name="work", bufs=1) as work:
            # masked-bg x-shift variants (host-prepared): index v holds
            # x[w + v - 1] (0 at edges); rows padded (34) for dy offsets.
            bgbT = work.tile([128, 3, 2, 34, 32], BF16, name="bgbT")
            bg8T = work.tile([128, 3, 2, 34, 32], F8, name="bg8T")
            fg8T = work.tile([128, 3, 2, 32, 32], F8, name="fg8T")
            bgb = [bgbT[:, v] for v in range(3)]
            bg8h = [bg8T[:, v] for v in range(3)]
            fg8h = [fg8T[:, v] for v in range(3)]
            bgT8 = [work.tile([128, 2, 2304], F8, name=f"bgT8{q}")
                    for q in range(4)]
            rdrow = work.tile([1, 1024], F32, name="rdrow")
            s2row = work.tile([1, 1024], F32, name="s2row")

            stage_cm = tc.tile_pool(name="stage", bufs=1)
            stage = stage_cm.__enter__()
            ps_acc_cm = tc.tile_pool(name="ps_acc", bufs=1, space="PSUM")
            ps_acc = ps_acc_cm.__enter__()
            ps_p1_cm = tc.tile_pool(name="ps_p1", bufs=1, space="PSUM")
            ps_p1 = ps_p1_cm.__enter__()
            ps_sc_cm = tc.tile_pool(name="ps_sc", bufs=3, space="PSUM")
            ps_sc = ps_sc_cm.__enter__()
            ps_tra_cm = tc.tile_pool(name="ps_tra", bufs=2, space="PSUM")
            ps_tra = ps_tra_cm.__enter__()

            q2 = stage.tile([128, 2, 32, 32], F32R, name="q2")
            onesf0 = stage.tile([128, 1], F32, name="onesf0")

            # ---------- phase 0: loads + edge zeros ----------
            nc.sync.dma_start(msrow[:], mask_d[:])
            nc.sync.dma_start(bg8T[:, :, :, 1:33, :], bg8_d[:])
            nc.sync.dma_start(fg8T[:], fg8_d[:])
            nc.sync.dma_start(bgbT[:, 1, :, 1:33, :], bgb_d[:])
            nc.sync.dma_start(fflat[:, 0], fg_d[0])
            nc.sync.dma_start(fflat[:, 1], fg_d[1])
            nc.sync.dma_start(w8h[:], w8h_d[:])
            nc.sync.dma_start(w8l[:], w8l_d[:])
            nc.sync.dma_start(biasb[:], b_d[:])

            nc.gpsimd.memset(onesf0[:], 1.0)
            # preload the sqrt table (phase1's only activation); the exp
            # table loads once right after the sqrt, under the scores.
            nc.scalar.activation(onesf0[0:1], onesf0[0:1], AF.Sqrt)
            nc.gpsimd.memset(onesf0[:], 1.0)

            # mask broadcast chain first in the DVE queue (head critical path)
            onesrowR = stage.tile([1, 128], F32R, name="onesrowR")
            nc.vector.memset(onesrowR[:].bitcast(F32), 1.0)
            msrowR = stage.tile([1, 1024], F32R, name="msrowR")
            nc.vector.tensor_copy(msrowR[:], msrow[:])
            idf = stage.tile([128, 128], F32, name="idf")
            make_identity(nc, idf[:])
            nc.gpsimd.tensor_copy(idR[:], idf[:])
            nc.gpsimd.tensor_copy(onesB[:], onesf0[:])
            nc.gpsimd.tensor_copy(ones_col[:], onesf0[:])
            for q in range(4):
                _ring_zero_pair(nc, Ap[q], eng=nc.gpsimd)
                _ring_zero_pair(nc, T8p[q], eng=nc.gpsimd)
            for w in W:
                _ring_zero(nc, w, eng=nc.gpsimd)
            nc.gpsimd.memset(bgbT[:, :, :, 0:34:33, :].bitcast(U16), 0)
            nc.gpsimd.memset(bg8T[:, :, :, 0:34:33, :].bitcast(U8), 0)
            nc.gpsimd.memset(bgbT[:, 0, :, 1:33, 0:1].bitcast(U16), 0)
            nc.gpsimd.memset(bgbT[:, 2, :, 1:33, 31:32].bitcast(U16), 0)
            # bgb x-variants from the loaded center (4x DVE copies)
            nc.vector.tensor_copy(bgbT[:, 0, :, 1:33, 1:32],
                                  bgbT[:, 1, :, 1:33, 0:31])
            nc.vector.tensor_copy(bgbT[:, 2, :, 1:33, 0:31],
                                  bgbT[:, 1, :, 1:33, 1:32])

            invmaskB = stage.tile([128, 32, 32], BF16, name="invmaskB")
            for h in range(2):
                psb = ps_p1.tile([128, 16, 32], F32, name="psb", tag="p1")
                nc.tensor.matmul(
                    psb[:].rearrange("p a b -> p (a b)"), onesrowR[:],
                    msrowR[:, 512 * h:512 * (h + 1)], start=True, stop=True)
                nc.vector.tensor_scalar(
                    out=invmaskB[:, 16 * h:16 * (h + 1), :], in0=psb[:],
                    scalar1=-1.0, scalar2=1.0, op0=ALU.mult, op1=ALU.add)

            # q2 = (masked bg)^2, feeding the patch-norm row sums
            nc.scalar.activation(q2[:, 0], bgb[1][:, 0, 1:33, :], AF.Square)
            nc.scalar.activation(q2[:, 1], bgb[1][:, 1, 1:33, :], AF.Square)

            # ---------- phase 1: patch norms + Gb (emitted before scores
            # so its PE matmuls run while the loads finish) ----
            rpad = stage.tile([1, 34, 34], BF16, name="rpad")
            rscr = stage.tile([1, 34, 34], BF16, name="rscr")
            _ring_zero(nc, rpad, eng=nc.gpsimd)
            _ring_zero(nc, rscr, eng=nc.gpsimd)
            sqrow = stage.tile([1, 1024], F32, name="sqrow")
            for ch in range(2):
                pr = ps_p1.tile([1, 512], F32, name="pr", tag="p1")
                for cb in range(2):
                    nc.tensor.matmul(
                        pr[:], ones_col[:],
                        q2[:, cb, 16 * ch:16 * ch + 16, :],
                        start=(cb == 0), stop=(cb == 1))
                nc.scalar.copy(rpad[:, 1 + 16 * ch:17 + 16 * ch, 1:33], pr[:])
            ssqrow = stage.tile([1, 32, 32], BF16, name="ssqrow")
            _boxsum(nc, rscr, rpad, ssqrow)
            urow = stage.tile([1, 1024], F32, name="urow")
            nc.vector.tensor_scalar_add(
                urow[:], ssqrow[:].rearrange("o a b -> o (a b)"),
                2304.0 * EPS * EPS)
            nc.scalar.activation(urow[:], urow[:], AF.Sqrt)
            # single switch to the exp table, hidden under early scores
            nc.scalar.activation(onesf0[0:1], onesf0[0:1], AF.Exp)
            nc.vector.reciprocal(urow[:], urow[:])
            # rncol[p, t] = rnrow[128t + p]: broadcast + diag extract
            Bb = stage.tile([128, 1024], F32, name="Bb")
            nc.gpsimd.partition_broadcast(Bb[:], urow[:])
            junk = stage.tile([128, 128], F32, name="junk")
            for t in range(8):
                nc.vector.scalar_tensor_tensor(
                    out=junk[:], in0=Bb[:, 128 * t:128 * (t + 1)],
                    scalar=1.0, in1=idf[:], op0=ALU.mult, op1=ALU.mult,
                    accum_out=rncol[:, t:t + 1])
            nc.vector.tensor_scalar(out=rnccol[:], in0=rncol[:], scalar1=TS,
                                    scalar2=256.0, op0=ALU.mult, op1=ALU.min)
            nc.vector.tensor_scalar_mul(ercol[:], rncol[:], EPS)

            # boxg: G = 3x3 box of channel-sum(fg); Gb = eps*box(G) bcast
            for ch in range(2):
                prg = ps_p1.tile([1, 512], F32, name="prg", tag="p1")
                for cb in range(2):
                    nc.tensor.matmul(
                        prg[:], onesB[:],
                        fflat[:, cb, 16 * ch:16 * ch + 16, :],
                        start=(cb == 0), stop=(cb == 1))
                nc.scalar.copy(rpad[:, 1 + 16 * ch:17 + 16 * ch, 1:33], prg[:])
            _boxsum(nc, rscr, rpad, ssqrow)
            nc.vector.tensor_copy(rpad[:, 1:33, 1:33], ssqrow[:])
            bbrow = stage.tile([1, 32, 32], BF16, name="bbrow")
            _boxsum(nc, rscr, rpad, bbrow)
            ebrowB = stage.tile([1, 1024], BF16, name="ebrowB")
            nc.vector.tensor_scalar_mul(
                ebrowB[:], bbrow[:].rearrange("o a b -> o (a b)"), EPS)
            nc.gpsimd.partition_broadcast(
                Gb.rearrange("p a b -> p (a b)"), ebrowB[:])

            # ---------- scores loop (fp8 DR, hi only) ----------
            # row-accumulators pair up in PSUM banks via partition offsets
            accD = ps_acc.tile([128, 512], F32, name="accD")
            accS = ps_acc.tile([128, 512], F32, name="accS")
            dAcc = [accD[64 * c:64 * c + 1, :] for c in range(2)]
            s2Acc = [accS[64 * c:64 * c + 1, :] for c in range(2)]

            def emit_scores(t, ch):
                psc = ps_sc.tile([128, 16, 32], F32, name="psc", tag="psc")
                for i, (dy, dx) in enumerate(TAPS):
                    fr = 16 * ch + dy - 1
                    r0, r1 = max(0, -fr), min(16, 32 - fr)
                    nc.tensor.matmul(
                        psc[:, r0:r1, :],
                        bg8h[dx][:, :, 4 * t + dy:4 * t + dy + 4, :],
                        fg8h[dx][:, :, fr + r0:fr + r1, :],
                        start=(i == 0), stop=(i == 8), perf_mode=DR)
                half = A[t][:, 1 + 16 * ch:17 + 16 * ch, 1:33]
                nc.scalar.copy(half, psc[:])

            hp_cm = tc.tile_pool(name="hp", bufs=2)
            hp = hp_cm.__enter__()

            def emit_boxsum_exp(t):
                w = W[t % 2]
                nc.vector.tensor_tensor(
                    w[:, 1:33, 1:33], A[t][:, 1:33, 0:32],
                    A[t][:, 1:33, 1:33], ALU.add)
                nc.vector.tensor_tensor(
                    w[:, 1:33, 1:33], w[:, 1:33, 1:33],
                    A[t][:, 1:33, 2:34], ALU.add)
                H = hp.tile([128, 32, 32], BF16, name="H", tag="H")
                nc.vector.tensor_tensor(
                    H[:], w[:, 0:32, 1:33], w[:, 1:33, 1:33], ALU.add)
                nc.vector.tensor_tensor(H[:], H[:], w[:, 2:34, 1:33], ALU.add)
                eng = nc.vector if t >= 6 else nc.gpsimd
                eng.tensor_tensor(H[:], H[:], Gb[:], ALU.add)
                for ch in range(2):
                    nc.scalar.activation(
                        A[t][:, 1 + 16 * ch:17 + 16 * ch, 1:33],
                        H[:, 16 * ch:16 * ch + 16, :],
                        AF.Exp, scale=rncol[:, t:t + 1])

            def emit_d_s2(t):
                for ch in range(2):
                    av = A[t][:, 1 + 16 * ch:17 + 16 * ch, 1:33]
                    nc.tensor.matmul(dAcc[ch][:], onesB[:], av,
                                     start=(t == 0), stop=(t == 7))
                    nc.tensor.matmul(s2Acc[ch][:], ercol[:, t:t + 1],
                                     av, start=(t == 0), stop=(t == 7))

            def build_bgT8(q):
                # transpose (j, cb, d) 128x128 blocks of shifted masked bg
                # into bf16 psum, 4 at a time, then fp8-quantize to SBUF.
                blocks = [(j, cb, d) for j in range(2) for cb in range(2)
                          for d in range(9)]
                for g in range(9):
                    chunk = blocks[4 * g:4 * g + 4]
                    ptr = ps_tra.tile([128, 512], BF16, name="ptr", tag="ptr")
                    for bi, (j, cb, d) in enumerate(chunk):
                        dy, dx = OFFS[d]
                        t = 2 * q + j
                        nc.tensor.transpose(
                            ptr[:, 128 * bi:128 * (bi + 1)],
                            bgb[dx][:, cb, 4 * t + dy:4 * t + dy + 4, :]
                            .rearrange("p a b -> p (a b)"),
                            idR[:])
                    dst = bgT8[q].rearrange("p j c -> p (j c)")[
                        :, 512 * g:512 * g + 512]
                    if g % 3 < 2:
                        nc.scalar.copy(dst, ptr[:])
                    else:
                        nc.vector.tensor_copy(dst, ptr[:])

            for t in range(8):
                emit_scores(t, 0)
                if t >= 3:
                    emit_d_s2(t - 3)
                emit_scores(t, 1)
                if t in (1, 2, 4, 5):
                    build_bgT8({1: 0, 2: 1, 4: 2, 5: 3}[t])
                emit_boxsum_exp(t)
            emit_d_s2(5)
            emit_d_s2(6)
            emit_d_s2(7)
            hp_cm.__exit__(None, None, None)
            ps_tra_cm.__exit__(None, None, None)
            ps_sc_cm.__exit__(None, None, None)
            ps_p1_cm.__exit__(None, None, None)

            # ---------- softmax denom + T8 + tconv ----------
            ps_db_cm = tc.tile_pool(name="ps_db", bufs=1, space="PSUM")
            ps_db = ps_db_cm.__enter__()
            psd = [ps_db.tile([128, 512], F32, name=f"psd{c}") for c in range(2)]
            rdrowR = stage.tile([1, 1024], F32R, name="rdrowR")
            Db = stage.tile([128, 32, 32], BF16, name="Db")
            for ch in range(2):
                nc.vector.reciprocal(
                    rdrow[:, 512 * ch:512 * (ch + 1)], dAcc[ch][:])
                nc.vector.tensor_copy(rdrowR[:, 512 * ch:512 * (ch + 1)],
                                      rdrow[:, 512 * ch:512 * (ch + 1)])
                nc.tensor.matmul(psd[ch][:], onesrowR[:],
                                 rdrowR[:, 512 * ch:512 * (ch + 1)],
                                 start=True, stop=True)
                nc.scalar.copy(
                    Db[:, 16 * ch:16 * ch + 16, :],
                    psd[ch][:].rearrange("p (a b) -> p a b", b=32))
            for ch in range(2):
                nc.vector.tensor_tensor(
                    s2row[:, 512 * ch:512 * (ch + 1)], s2Acc[ch][:],
                    rdrow[:, 512 * ch:512 * (ch + 1)], ALU.mult)

            def emit_T8(t):
                q, j = t // 2, t % 2
                if t < 2:
                    for ch in range(2):
                        nc.vector.scalar_tensor_tensor(
                            out=T8p[q][:, j, 1 + 16 * ch:17 + 16 * ch, 1:33],
                            in0=A[t][:, 1 + 16 * ch:17 + 16 * ch, 1:33],
                            scalar=rnccol[:, t:t + 1],
                            in1=Db[:, 16 * ch:16 * ch + 16, :],
                            op0=ALU.mult, op1=ALU.mult)
                else:
                    nc.vector.scalar_tensor_tensor(
                        out=T8p[q][:, j, 1:33, 1:33],
                        in0=A[t][:, 1:33, 1:33],
                        scalar=rnccol[:, t:t + 1],
                        in1=Db[:],
                        op0=ALU.mult, op1=ALU.mult)

            # mask constants (needed by the final assembly only)
            nc.gpsimd.tensor_copy(invmaskb[:], invmaskB[:])
            nc.gpsimd.tensor_scalar(
                out=maskb9[:], in0=invmaskB[:], scalar1=-1.0 / 9.0,
                scalar2=1.0 / 9.0, op0=ALU.mult, op1=ALU.add)
            nc.gpsimd.tensor_scalar(
                out=maskb9s[:], in0=invmaskB[:], scalar1=-1.0 / (9.0 * TS),
                scalar2=1.0 / (9.0 * TS), op0=ALU.mult, op1=ALU.add)

            # s2 path early: runs on Pool while T8/tconv proceed
            s2pad = W[0][0:1]
            s2scr = W[1][0:1]
            _ring_zero(nc, s2pad, eng=nc.gpsimd)
            _ring_zero(nc, s2scr, eng=nc.gpsimd)
            nc.gpsimd.tensor_copy(
                s2pad[:, 1:33, 1:33],
                s2row[:].rearrange("o (a b) -> o a b", b=32))
            boxs2 = stage.tile([1, 32, 32], BF16, name="boxs2")
            _boxsum(nc, s2scr, s2pad, boxs2, eng=nc.gpsimd)
            nc.gpsimd.tensor_scalar_mul(
                s2row[:], boxs2[:].rearrange("o a b -> o (a b)"), EPS)
            s2b = stage.tile([128, 32, 32], F32, name="s2b")
            nc.gpsimd.partition_broadcast(
                s2b.rearrange("p a b -> p (a b)"), s2row[:])
            nc.gpsimd.tensor_tensor(s2b[:], s2b[:], maskb9[:], ALU.mult)

            with tc.tile_pool(name="ps_rec", bufs=1, space="PSUM") as psrec:
                prec = [[psrec.tile([128, 512], F32, name=f"prec{c}_{ch}")
                         for ch in range(2)] for c in range(2)]

                def emit_tconv(q):
                    for cb in range(2):
                        for ch in range(2):
                            for d, (dy, dx) in enumerate(OFFS):
                                z0 = 16 * ch + 2 - dy
                                x0 = 2 - dx
                                nc.tensor.matmul(
                                    prec[cb][ch][:],
                                    bgT8[q][:, :, 128 * (9 * cb + d):
                                            128 * (9 * cb + d + 1)],
                                    T8p[q][:, :, z0:z0 + 16, x0:x0 + 32],
                                    start=(q == 0 and d == 0),
                                    stop=(q == 3 and d == 8), perf_mode=DR)

                for t in range(8):
                    emit_T8(t)
                for q in range(4):
                    emit_tconv(q)

                # pre = eps*boxs2*mask/9 + fg*(1-mask): ready during tconv
                final_pad = [main.tile([128, 48, 48], BF16,
                                       name=f"final_pad{c}") for c in range(2)]
                pre = [stage.tile([128, 32, 32], F32, name=f"pre{c}")
                       for c in range(2)]
                for c in range(2):
                    nc.gpsimd.tensor_tensor(pre[c][:], fflat[:, c],
                                            invmaskb[:], ALU.mult)
                    nc.gpsimd.tensor_tensor(pre[c][:], pre[c][:], s2b[:],
                                            ALU.add)
                    nc.gpsimd.memset(final_pad[c][:, 0:8, :].bitcast(U16), 0)
                    nc.gpsimd.memset(final_pad[c][:, 40:48, :].bitcast(U16), 0)
                    nc.gpsimd.memset(
                        final_pad[c][:, 8:40, 0:8].bitcast(U16), 0)
                    nc.gpsimd.memset(
                        final_pad[c][:, 8:40, 40:48].bitcast(U16), 0)

                fscr = [stage.tile([128, 32, 32], F32, name=f"fscr{i}")
                        for i in range(2)]
                # assembly per (cb, ch) half; fp8 hi/lo quantize split at
                # row 28 so the r<=4 ch0 conv groups start on chunk a
                for cb in range(2):
                    for ch in range(2):
                        r0 = 16 * ch
                        nc.vector.tensor_tensor(
                            fscr[cb][:, r0:r0 + 16, :], prec[cb][ch][:],
                            maskb9s[:, r0:r0 + 16, :], ALU.mult)
                        nc.gpsimd.tensor_tensor(
                            final_pad[cb][:, 8 + r0:24 + r0, 8:40],
                            fscr[cb][:, r0:r0 + 16, :],
                            pre[cb][:, r0:r0 + 16, :], ALU.add)
                for cb in range(2):
                    nc.scalar.copy(f8h[:, cb, 0:28, :],
                                   final_pad[cb][:, 0:28, :])
                for cb in range(2):
                    nc.vector.scalar_tensor_tensor(
                        out=f8l[:, cb, 0:28, :],
                        in0=f8h[:, cb, 0:28, :], scalar=-1.0,
                        in1=final_pad[cb][:, 0:28, :],
                        op0=ALU.mult, op1=ALU.add)
                for cb in range(2):
                    nc.scalar.copy(f8h[:, cb, 28:48, :],
                                   final_pad[cb][:, 28:48, :])
                for cb in range(2):
                    nc.vector.scalar_tensor_tensor(
                        out=f8l[:, cb, 28:48, :],
                        in0=f8h[:, cb, 28:48, :], scalar=-1.0,
                        in1=final_pad[cb][:, 28:48, :],
                        op0=ALU.mult, op1=ALU.add)
            ps_db_cm.__exit__(None, None, None)
            ps_acc_cm.__exit__(None, None, None)
            stage_cm.__exit__(None, None, None)
        # ---------- work pool closed ----------

        with tc.tile_pool(name="late", bufs=1) as late:
            # ---------- dilated convs (fp8 DR, 3-term compensated) ----------
            out_sb = [late.tile([64, 32, 32], F32, name=f"out_sb{g}")
                      for g in range(4)]

            with tc.tile_pool(name="ps_o", bufs=4, space="PSUM") as pso_pool:
                for g, ch in ((0, 0), (1, 0), (2, 0), (0, 1), (1, 1),
                              (2, 1), (3, 0), (3, 1)):
                    if True:
                        r = RATES[g]
                        pso = pso_pool.tile([64, 512], F32, name="pso",
                                            tag="pso")
                        i = 0
                        for wt, ft in ((w8h, f8h), (w8h, f8l), (w8l, f8h)):
                            for d, (dy, dx) in enumerate(OFFS):
                                oy = 8 + r * (dy - 1) + 16 * ch
                                ox = 8 + r * (dx - 1)
                                woff = 576 * g + 64 * (3 * dy + dx)
                                nc.tensor.matmul(
                                    pso[:],
                                    wt[:, :, woff:woff + 64],
                                    ft[:, :, oy:oy + 16, ox:ox + 32],
                                    start=(i == 0), stop=(i == 26),
                                    perf_mode=DR)
                                i += 1
                        # split the last chunk so its DMA overlaps the relu
                        nsub = 2 if (g == 3 and ch == 1) else 1
                        for s in range(nsub):
                            rs = 16 * ch + (16 // nsub) * s
                            rn_ = 16 // nsub
                            nc.scalar.activation(
                                out_sb[g][:, rs:rs + rn_, :],
                                pso[:].rearrange("p (a b) -> p a b", b=32)
                                [:, rs - 16 * ch:rs - 16 * ch + rn_, :],
                                AF.Relu, bias=biasb[:, g:g + 1],
                                scale=1.0 / 32.0)
                            nc.sync.dma_start(
                                out_d[64 * g:64 * (g + 1), rs:rs + rn_, :],
                                out_sb[g][:, rs:rs + rn_, :])


def _get_nc():
    if "nc" not in _CACHE:
        _CACHE["nc"] = build_program()
    return _CACHE["nc"]


def kernel(foreground, mask, background, conv_w, conv_b):
    nc = _get_nc()
    fg = np.ascontiguousarray(foreground, dtype=np.float32).reshape(
        8, 2, 128, 32, 32).astype(ml_dtypes.bfloat16)

    def xvariants(x):
        # [8, 2, 128, 32, 32] -> [8, 128, 3, 2, 32, 32], v holds x+v-1
        z = np.zeros_like(x[..., :1])
        v = np.stack([
            np.concatenate([z, x[..., :-1]], axis=-1),
            x,
            np.concatenate([x[..., 1:], z], axis=-1),
        ], axis=1)                                   # [8, 3, 2, 128, 32, 32]
        return np.ascontiguousarray(v.transpose(0, 3, 1, 2, 4, 5))

    fg8 = xvariants(fg.astype(ml_dtypes.float8_e4m3))
    bgm = (np.ascontiguousarray(background, dtype=np.float32).reshape(
        8, 2, 128, 32, 32) * (1.0 - mask.reshape(1, 1, 1, 32, 32))
    ).astype(ml_dtypes.bfloat16)
    bgmc = np.ascontiguousarray(bgm.transpose(0, 2, 1, 3, 4))
    bg8 = xvariants(bgm.astype(ml_dtypes.float8_e4m3))
    maskrow = np.ascontiguousarray(mask.reshape(1, 1024), dtype=np.float32)
    # conv_w [4,64,256,3,3] -> [k, cb, g*9*64] fp8 hi/lo pair, x32 gain
    w32 = np.ascontiguousarray(
        conv_w.astype(np.float32).transpose(2, 0, 3, 4, 1).reshape(2, 128, 2304)
    ).transpose(1, 0, 2) * 32.0
    w8h = np.ascontiguousarray(w32).astype(ml_dtypes.float8_e4m3)
    w8l = np.ascontiguousarray(
        w32 - w8h.astype(np.float32)).astype(ml_dtypes.float8_e4m3)
    bias = np.ascontiguousarray(conv_b.astype(np.float32).reshape(4, 64).T)
    in_maps = [
        {"fg": fg[i], "fg8": fg8[i], "bgm": bgmc[i], "bg8": bg8[i],
         "maskrow": maskrow, "w8h": w8h, "w8l": w8l, "bias": bias}
        for i in range(8)
    ]
    res = run_bass_kernel_spmd(nc, in_maps, list(range(8)))
    return np.stack([res.results[i]["out"] for i in range(8)], axis=0)


if __name__ == "__main__":
    build_program()
    print("build ok")


# revision 98
# speedup vs baseline: 1.0239x; 1.0239x over previous
"""Trainium2 Bass kernel for ContextualAttentionModule.

Data-parallel over batch: 8 samples -> 8 NeuronCores, one sample per core.
Per-core pipeline (C=256, H=W=32, L=1024 patches):
  scores  = <fg_patch(p), bg_patch(l)> via fp8e4 DoubleRow matmuls, hi-term
            only (the recovered/attention branch carries ~3% of output
            energy, so fp8 rounding there is far under the error budget)
  norm    = rsqrt(3x3-boxsum(colsum bg_masked^2) + 2304*eps^2), bf16 boxsum
  prop    = 3x3 window-sum of scores + eps*boxbox(colsum fg)  (DVE, bf16)
  E       = exp(prop * rn) in bf16 (no max-subtract)
  D, s2   = interleaved per-block [1,512] PE accumulations over E (lag 2)
  T8      = fp8(E * (1/D) * min(rn*2048, 256))  -- the 2^11 gain keeps the
            attn operand in fp8's normal range; undone exactly via the
            mask/(9*2048) constant tile.  1/D read directly from PSUM.
  recov   = fp8 DoubleRow tconv: t-block pairs contract in one matmul
            (lhs = transposed masked-bg banks quantized to fp8, rhs = T8)
  final   = (recov*mask/(9*2048)*2048 + eps*box(s2/D))*... + fg*(1-mask)
  out     = concat_g relu(dilated_conv_r(final) + b)  bf16 weights
"""

import numpy as np
import ml_dtypes

import concourse.bass as bass
import concourse.tile as tile
from concourse import bacc, mybir
from concourse.bass_utils import run_bass_kernel_spmd
from concourse.masks import make_identity

F32 = mybir.dt.float32
F32R = mybir.dt.float32r
BF16 = mybir.dt.bfloat16
F8 = mybir.dt.float8e4
U8 = mybir.dt.uint8
U16 = mybir.dt.uint16
AF = mybir.ActivationFunctionType
ALU = mybir.AluOpType
DR = mybir.MatmulPerfMode.DoubleRow

EPS = 1e-7
TS = 2048.0          # power-of-2 gain on the attn operand for fp8 range
RATES = (1, 2, 4, 8)
OFFS = [(dy, dx) for dy in range(3) for dx in range(3)]
# dy=1 taps first so the start=True matmul covers the full psum tile
TAPS = [(1, 0), (1, 1), (1, 2), (0, 0), (0, 1), (0, 2), (2, 0), (2, 1), (2, 2)]

_CACHE = {}


def _ring_zero(nc, buf, n=34, eng=None):
    """Zero only the 1-wide border ring of a [P, n, n] padded buffer."""
    eng = eng or nc.vector
    bc = U16 if buf.dtype == BF16 else F32
    eng.memset(buf[:, 0:n:n - 1, :].bitcast(bc), 0)
    eng.memset(buf[:, 1:n - 1, 0:n:n - 1].bitcast(bc), 0)


def _ring_zero_pair(nc, buf, eng=None):
    """Ring-zero both j-halves of a [P, 2, 34, 34] paired buffer."""
    eng = eng or nc.vector
    bc = U16 if buf.dtype == BF16 else U8
    eng.memset(buf[:, :, 0:34:33, :].bitcast(bc), 0)
    eng.memset(buf[:, :, 1:33, 0:34:33].bitcast(bc), 0)


def _boxsum(nc, scr, src_pad, dst_flat, eng=None):
    """3x3 SAME window sum: [1,34,34] ring-zero padded -> [1,32,32] flat."""
    eng = eng or nc.vector
    eng.tensor_tensor(scr[:, 1:33, 1:33], src_pad[:, 1:33, 0:32],
                      src_pad[:, 1:33, 1:33], ALU.add)
    eng.tensor_tensor(scr[:, 1:33, 1:33], scr[:, 1:33, 1:33],
                      src_pad[:, 1:33, 2:34], ALU.add)
    eng.tensor_tensor(dst_flat[:], scr[:, 0:32, 1:33],
                      scr[:, 1:33, 1:33], ALU.add)
    eng.tensor_tensor(dst_flat[:], dst_flat[:], scr[:, 2:34, 1:33], ALU.add)


def build_program():
    nc = bacc.Bacc()
    fg_d = nc.declare_dram_parameter("fg", [2, 128, 32, 32], BF16, isOutput=False)
    fg8_d = nc.declare_dram_parameter("fg8", [128, 3, 2, 32, 32], F8,
                                      isOutput=False)
    bgb_d = nc.declare_dram_parameter("bgm", [128, 2, 32, 32], BF16,
                                      isOutput=False)
    bg8_d = nc.declare_dram_parameter("bg8", [128, 3, 2, 32, 32], F8,
                                      isOutput=False)
    mask_d = nc.declare_dram_parameter("maskrow", [1, 1024], F32, isOutput=False)
    w8h_d = nc.declare_dram_parameter("w8h", [128, 2, 2304], F8, isOutput=False)
    w8l_d = nc.declare_dram_parameter("w8l", [128, 2, 2304], F8, isOutput=False)
    b_d = nc.declare_dram_parameter("bias", [64, 4], F32, isOutput=False)
    out_d = nc.declare_dram_parameter("out", [256, 32, 32], F32, isOutput=True)

    with tile.TileContext(nc) as tc:
        _emit(nc, tc, fg_d, fg8_d, bgb_d, bg8_d, mask_d, w8h_d, w8l_d,
              b_d, out_d)
    nc.compile()
    return nc


def _emit(nc, tc, fg_d, fg8_d, bgb_d, bg8_d, mask_d, w8h_d, w8l_d,
          b_d, out_d):
    with tc.tile_pool(name="main", bufs=1) as main:
        # ---------------- long-lived tiles ----------------
        fflat = main.tile([128, 2, 32, 32], BF16, name="fflat")
        invmaskb = main.tile([128, 32, 32], F32, name="invmaskb")
        maskb9 = main.tile([128, 32, 32], F32, name="maskb9")
        maskb9s = main.tile([128, 32, 32], F32, name="maskb9s")
        ones_col = main.tile([128, 1], F32R, name="ones_col")
        idR = main.tile([128, 128], BF16, name="idR")
        rncol = main.tile([128, 8], F32, name="rncol")
        rnccol = main.tile([128, 8], F32, name="rnccol")
        ercol = main.tile([128, 8], BF16, name="ercol")
        onesB = main.tile([128, 1], BF16, name="onesB")
        Gb = main.tile([128, 32, 32], BF16, name="Gb")
        msrow = main.tile([1, 1024], F32, name="msrow")
        w8h = main.tile([128, 2, 2304], F8, name="w8h")
        w8l = main.tile([128, 2, 2304], F8, name="w8l")
        f8h = main.tile([128, 2, 48, 48], F8, name="f8h")
        f8l = main.tile([128, 2, 48, 48], F8, name="f8l")
        biasb = main.tile([64, 4], F32, name="biasb")
        Ap = [main.tile([128, 2, 34, 34], BF16, name=f"Ap{q}") for q in range(4)]
        A = [Ap[t // 2][:, t % 2] for t in range(8)]
        T8p = [main.tile([128, 2, 34, 34], F8, name=f"T8p{q}") for q in range(4)]
        W = [main.tile([128, 34, 34], BF16, name=f"W{i}") for i in range(2)]

        with tc.tile_pool(name="work", bufs=1) as work:
            # masked-bg x-shift variants (host-prepared): index v holds
            # x[w + v - 1] (0 at edges); rows padded (34) for dy offsets.
            bgbT = work.tile([128, 3, 2, 34, 32], BF16, name="bgbT")
            bg8T = work.tile([128, 3, 2, 34, 32], F8, name="bg8T")
            fg8T = work.tile([128, 3, 2, 32, 32], F8, name="fg8T")
            bgb = [bgbT[:, v] for v in range(3)]
            bg8h = [bg8T[:, v] for v in range(3)]
            fg8h = [fg8T[:, v] for v in range(3)]
            bgT8 = [work.tile([128, 2, 2304], F8, name=f"bgT8{q}")
                    for q in range(4)]
            rdrow = work.tile([1, 1024], F32, name="rdrow")
            s2row = work.tile([1, 1024], F32, name="s2row")

            stage_cm = tc.tile_pool(name="stage", bufs=1)
            stage = stage_cm.__enter__()
            ps_acc_cm = tc.tile_pool(name="ps_acc", bufs=1, space="PSUM")
            ps_acc = ps_acc_cm.__enter__()
            ps_p1_cm = tc.tile_pool(name="ps_p1", bufs=1, space="PSUM")
            ps_p1 = ps_p1_cm.__enter__()
            ps_sc_cm = tc.tile_pool(name="ps_sc", bufs=3, space="PSUM")
            ps_sc = ps_sc_cm.__enter__()
            ps_tra_cm = tc.tile_pool(name="ps_tra", bufs=2, space="PSUM")
            ps_tra = ps_tra_cm.__enter__()

            q2 = stage.tile([128, 2, 32, 32], F32R, name="q2")
            onesf0 = stage.tile([128, 1], F32, name="onesf0")

            # ---------- phase 0: loads + edge zeros ----------
            nc.sync.dma_start(msrow[:], mask_d[:])
            nc.sync.dma_start(bg8T[:, :, :, 1:33, :], bg8_d[:])
            nc.sync.dma_start(fg8T[:], fg8_d[:])
            nc.sync.dma_start(bgbT[:, 1, :, 1:33, :], bgb_d[:])
            nc.sync.dma_start(fflat[:, 0], fg_d[0])
            nc.sync.dma_start(fflat[:, 1], fg_d[1])
            nc.sync.dma_start(w8h[:], w8h_d[:])
            nc.sync.dma_start(w8l[:], w8l_d[:])
            nc.sync.dma_start(biasb[:], b_d[:])

            nc.gpsimd.memset(onesf0[:], 1.0)
            # preload the sqrt table (phase1's only activation); the exp
            # table loads once right after the sqrt, under the scores.
            nc.scalar.activation(onesf0[0:1], onesf0[0:1], AF.Sqrt)
            nc.gpsimd.memset(onesf0[:], 1.0)

            # mask broadcast chain first in the DVE queue (head critical path)
            onesrowR = stage.tile([1, 128], F32R, name="onesrowR")
            nc.vector.memset(onesrowR[:].bitcast(F32), 1.0)
            msrowR = stage.tile([1, 1024], F32R, name="msrowR")
            nc.vector.tensor_copy(msrowR[:], msrow[:])
            idf = stage.tile([128, 128], F32, name="idf")
            make_identity(nc, idf[:])
            nc.gpsimd.tensor_copy(idR[:], idf[:])
            nc.gpsimd.tensor_copy(onesB[:], onesf0[:])
            nc.gpsimd.tensor_copy(ones_col[:], onesf0[:])
            for q in range(4):
                _ring_zero_pair(nc, Ap[q], eng=nc.gpsimd)
                _ring_zero_pair(nc, T8p[q], eng=nc.gpsimd)
            for w in W:
                _ring_zero(nc, w, eng=nc.gpsimd)
            nc.gpsimd.memset(bgbT[:, :, :, 0:34:33, :].bitcast(U16), 0)
            nc.gpsimd.memset(bg8T[:, :, :, 0:34:33, :].bitcast(U8), 0)
            nc.gpsimd.memset(bgbT[:, 0, :, 1:33, 0:1].bitcast(U16), 0)
            nc.gpsimd.memset(bgbT[:, 2, :, 1:33, 31:32].bitcast(U16), 0)
            # bgb x-variants from the loaded center (4x DVE copies)
            nc.vector.tensor_copy(bgbT[:, 0, :, 1:33, 1:32],
                                  bgbT[:, 1, :, 1:33, 0:31])
            nc.vector.tensor_copy(bgbT[:, 2, :, 1:33, 0:31],
                                  bgbT[:, 1, :, 1:33, 1:32])

            invmaskB = stage.tile([128, 32, 32], BF16, name="invmaskB")
            for h in range(2):
                psb = ps_p1.tile([128, 16, 32], F32, name="psb", tag="p1")
                nc.tensor.matmul(
                    psb[:].rearrange("p a b -> p (a b)"), onesrowR[:],
                    msrowR[:, 512 * h:512 * (h + 1)], start=True, stop=True)
                nc.vector.tensor_scalar(
                    out=invmaskB[:, 16 * h:16 * (h + 1), :], in0=psb[:],
                    scalar1=-1.0, scalar2=1.0, op0=ALU.mult, op1=ALU.add)

            # q2 = (masked bg)^2, feeding the patch-norm row sums
            nc.scalar.activation(q2[:, 0], bgb[1][:, 0, 1:33, :], AF.Square)
            nc.scalar.activation(q2[:, 1], bgb[1][:, 1, 1:33, :], AF.Square)

            # ---------- phase 1: patch norms + Gb (emitted before scores
            # so its PE matmuls run while the loads finish) ----
            rpad = stage.tile([1, 34, 34], BF16, name="rpad")
            rscr = stage.tile([1, 34, 34], BF16, name="rscr")
            _ring_zero(nc, rpad, eng=nc.gpsimd)
            _ring_zero(nc, rscr, eng=nc.gpsimd)
            sqrow = stage.tile([1, 1024], F32, name="sqrow")
            for ch in range(2):
                pr = ps_p1.tile([1, 512], F32, name="pr", tag="p1")
                for cb in range(2):
                    nc.tensor.matmul(
                        pr[:], ones_col[:],
                        q2[:, cb, 16 * ch:16 * ch + 16, :],
                        start=(cb == 0), stop=(cb == 1))
                nc.scalar.copy(rpad[:, 1 + 16 * ch:17 + 16 * ch, 1:33], pr[:])
            ssqrow = stage.tile([1, 32, 32], BF16, name="ssqrow")
            _boxsum(nc, rscr, rpad, ssqrow)
            urow = stage.tile([1, 1024], F32, name="urow")
            nc.vector.tensor_scalar_add(
                urow[:], ssqrow[:].rearrange("o a b -> o (a b)"),
                2304.0 * EPS * EPS)
            nc.scalar.activation(urow[:], urow[:], AF.Sqrt)
            # single switch to the exp table, hidden under early scores
            nc.scalar.activation(onesf0[0:1], onesf0[0:1], AF.Exp)
            nc.vector.reciprocal(urow[:], urow[:])
            # rncol[p, t] = rnrow[128t + p]: broadcast + diag extract
            Bb = stage.tile([128, 1024], F32, name="Bb")
            nc.gpsimd.partition_broadcast(Bb[:], urow[:])
            junk = stage.tile([128, 128], F32, name="junk")
            for t in range(8):
                nc.vector.scalar_tensor_tensor(
                    out=junk[:], in0=Bb[:, 128 * t:128 * (t + 1)],
                    scalar=1.0, in1=idf[:], op0=ALU.mult, op1=ALU.mult,
                    accum_out=rncol[:, t:t + 1])
            nc.vector.tensor_scalar(out=rnccol[:], in0=rncol[:], scalar1=TS,
                                    scalar2=256.0, op0=ALU.mult, op1=ALU.min)
            nc.vector.tensor_scalar_mul(ercol[:], rncol[:], EPS)

            # boxg: G = 3x3 box of channel-sum(fg); Gb = eps*box(G) bcast
            for ch in range(2):
                prg = ps_p1.tile([1, 512], F32, name="prg", tag="p1")
                for cb in range(2):
                    nc.tensor.matmul(
                        prg[:], onesB[:],
                        fflat[:, cb, 16 * ch:16 * ch + 16, :],
                        start=(cb == 0), stop=(cb == 1))
                nc.scalar.copy(rpad[:, 1 + 16 * ch:17 + 16 * ch, 1:33], prg[:])
            _boxsum(nc, rscr, rpad, ssqrow)
            nc.vector.tensor_copy(rpad[:, 1:33, 1:33], ssqrow[:])
            bbrow = stage.tile([1, 32, 32], BF16, name="bbrow")
            _boxsum(nc, rscr, rpad, bbrow)
            ebrowB = stage.tile([1, 1024], BF16, name="ebrowB")
            nc.vector.tensor_scalar_mul(
                ebrowB[:], bbrow[:].rearrange("o a b -> o (a b)"), EPS)
            nc.gpsimd.partition_broadcast(
                Gb.rearrange("p a b -> p (a b)"), ebrowB[:])

            # ---------- scores loop (fp8 DR, hi only) ----------
            # row-accumulators pair up in PSUM banks via partition offsets
            accD = ps_acc.tile([128, 512], F32, name="accD")
            accS = ps_acc.tile([128, 512], F32, name="accS")
            dAcc = [accD[64 * c:64 * c + 1, :] for c in range(2)]
            s2Acc = [accS[64 * c:64 * c + 1, :] for c in range(2)]

            def emit_scores(t, ch):
                psc = ps_sc.tile([128, 16, 32], F32, name="psc", tag="psc")
                for i, (dy, dx) in enumerate(TAPS):
                    fr = 16 * ch + dy - 1
                    r0, r1 = max(0, -fr), min(16, 32 - fr)
                    nc.tensor.matmul(
                        psc[:, r0:r1, :],
                        bg8h[dx][:, :, 4 * t + dy:4 * t + dy + 4, :],
                        fg8h[dx][:, :, fr + r0:fr + r1, :],
                        start=(i == 0), stop=(i == 8), perf_mode=DR)
                half = A[t][:, 1 + 16 * ch:17 + 16 * ch, 1:33]
                nc.scalar.copy(half, psc[:])

            hp_cm = tc.tile_pool(name="hp", bufs=2)
            hp = hp_cm.__enter__()

            def emit_boxsum_exp(t):
                w = W[t % 2]
                nc.vector.tensor_tensor(
                    w[:, 1:33, 1:33], A[t][:, 1:33, 0:32],
                    A[t][:, 1:33, 1:33], ALU.add)
                nc.vector.tensor_tensor(
                    w[:, 1:33, 1:33], w[:, 1:33, 1:33],
                    A[t][:, 1:33, 2:34], ALU.add)
                H = hp.tile([128, 32, 32], BF16, name="H", tag="H")
                nc.vector.tensor_tensor(
                    H[:], w[:, 0:32, 1:33], w[:, 1:33, 1:33], ALU.add)
                nc.vector.tensor_tensor(H[:], H[:], w[:, 2:34, 1:33], ALU.add)
                eng = nc.vector if t >= 6 else nc.gpsimd
                eng.tensor_tensor(H[:], H[:], Gb[:], ALU.add)
                for ch in range(2):
                    nc.scalar.activation(
                        A[t][:, 1 + 16 * ch:17 + 16 * ch, 1:33],
                        H[:, 16 * ch:16 * ch + 16, :],
                        AF.Exp, scale=rncol[:, t:t + 1])

            def emit_d_s2(t):
                for ch in range(2):
                    av = A[t][:, 1 + 16 * ch:17 + 16 * ch, 1:33]
                    nc.tensor.matmul(dAcc[ch][:], onesB[:], av,
                                     start=(t == 0), stop=(t == 7))
                    nc.tensor.matmul(s2Acc[ch][:], ercol[:, t:t + 1],
                                     av, start=(t == 0), stop=(t == 7))

            def build_bgT8(q):
                # transpose (j, cb, d) 128x128 blocks of shifted masked bg
                # into bf16 psum, 4 at a time, then fp8-quantize to SBUF.
                blocks = [(j, cb, d) for j in range(2) for cb in range(2)
                          for d in range(9)]
                for g in range(9):
                    chunk = blocks[4 * g:4 * g + 4]
                    ptr = ps_tra.tile([128, 512], BF16, name="ptr", tag="ptr")
                    for bi, (j, cb, d) in enumerate(chunk):
                        dy, dx = OFFS[d]
                        t = 2 * q + j
                        nc.tensor.transpose(
                            ptr[:, 128 * bi:128 * (bi + 1)],
                            bgb[dx][:, cb, 4 * t + dy:4 * t + dy + 4, :]
                            .rearrange("p a b -> p (a b)"),
                            idR[:])
                    dst = bgT8[q].rearrange("p j c -> p (j c)")[
                        :, 512 * g:512 * g + 512]
                    if g % 3 < 2:
                        nc.scalar.copy(dst, ptr[:])
                    else:
                        nc.vector.tensor_copy(dst, ptr[:])

            for t in range(8):
                emit_scores(t, 0)
                if t >= 3:
                    emit_d_s2(t - 3)
                emit_scores(t, 1)
                if t in (1, 2, 4, 5):
                    build_bgT8({1: 0, 2: 1, 4: 2, 5: 3}[t])
                emit_boxsum_exp(t)
            emit_d_s2(5)
            emit_d_s2(6)
            emit_d_s2(7)
            hp_cm.__exit__(None, None, None)
            ps_tra_cm.__exit__(None, None, None)
            ps_sc_cm.__exit__(None, None, None)
            ps_p1_cm.__exit__(None, None, None)

            # ---------- softmax denom + T8 + tconv ----------
            ps_db_cm = tc.tile_pool(name="ps_db", bufs=1, space="PSUM")
            ps_db = ps_db_cm.__enter__()
            psd = [ps_db.tile([128, 512], F32, name=f"psd{c}") for c in range(2)]
            rdrowR = stage.tile([1, 1024], F32R, name="rdrowR")
            Db = stage.tile([128, 32, 32], BF16, name="Db")
            for ch in range(2):
                nc.vector.reciprocal(
                    rdrow[:, 512 * ch:512 * (ch + 1)], dAcc[ch][:])
                nc.vector.tensor_copy(rdrowR[:, 512 * ch:512 * (ch + 1)],
                                      rdrow[:, 512 * ch:512 * (ch + 1)])
                nc.tensor.matmul(psd[ch][:], onesrowR[:],
                                 rdrowR[:, 512 * ch:512 * (ch + 1)],
                                 start=True, stop=True)
                nc.scalar.copy(
                    Db[:, 16 * ch:16 * ch + 16, :],
                    psd[ch][:].rearrange("p (a b) -> p a b", b=32))
            for ch in range(2):
                nc.vector.tensor_tensor(
                    s2row[:, 512 * ch:512 * (ch + 1)], s2Acc[ch][:],
                    rdrow[:, 512 * ch:512 * (ch + 1)], ALU.mult)

            def emit_T8(t):
                q, j = t // 2, t % 2
                if t < 2:
                    for ch in range(2):
                        nc.vector.scalar_tensor_tensor(
                            out=T8p[q][:, j, 1 + 16 * ch:17 + 16 * ch, 1:33],
                            in0=A[t][:, 1 + 16 * ch:17 + 16 * ch, 1:33],
                            scalar=rnccol[:, t:t + 1],
                            in1=Db[:, 16 * ch:16 * ch + 16, :],
                            op0=ALU.mult, op1=ALU.mult)
                else:
                    nc.vector.scalar_tensor_tensor(
                        out=T8p[q][:, j, 1:33, 1:33],
                        in0=A[t][:, 1:33, 1:33],
                        scalar=rnccol[:, t:t + 1],
                        in1=Db[:],
                        op0=ALU.mult, op1=ALU.mult)

            # mask constants (needed by the final assembly only)
            nc.gpsimd.tensor_copy(invmaskb[:], invmaskB[:])
            nc.gpsimd.tensor_scalar(
                out=maskb9[:], in0=invmaskB[:], scalar1=-1.0 / 9.0,
                scalar2=1.0 / 9.0, op0=ALU.mult, op1=ALU.add)
            nc.gpsimd.tensor_scalar(
                out=maskb9s[:], in0=invmaskB[:], scalar1=-1.0 / (9.0 * TS),
                scalar2=1.0 / (9.0 * TS), op0=ALU.mult, op1=ALU.add)

            # s2 path early: runs on Pool while T8/tconv proceed
            s2pad = W[0][0:1]
            s2scr = W[1][0:1]
            _ring_zero(nc, s2pad, eng=nc.gpsimd)
            _ring_zero(nc, s2scr, eng=nc.gpsimd)
            nc.gpsimd.tensor_copy(
                s2pad[:, 1:33, 1:33],
                s2row[:].rearrange("o (a b) -> o a b", b=32))
            boxs2 = stage.tile([1, 32, 32], BF16, name="boxs2")
            _boxsum(nc, s2scr, s2pad, boxs2, eng=nc.gpsimd)
            nc.gpsimd.tensor_scalar_mul(
                s2row[:], boxs2[:].rearrange("o a b -> o (a b)"), EPS)
            s2b = stage.tile([128, 32, 32], F32, name="s2b")
            nc.gpsimd.partition_broadcast(
                s2b.rearrange("p a b -> p (a b)"), s2row[:])
            nc.gpsimd.tensor_tensor(s2b[:], s2b[:], maskb9[:], ALU.mult)

            with tc.tile_pool(name="ps_rec", bufs=1, space="PSUM") as psrec:
                prec = [[psrec.tile([128, 512], F32, name=f"prec{c}_{ch}")
                         for ch in range(2)] for c in range(2)]

                def emit_tconv(q):
                    for cb in range(2):
                        for ch in range(2):
                            for d, (dy, dx) in enumerate(OFFS):
                                z0 = 16 * ch + 2 - dy
                                x0 = 2 - dx
                                nc.tensor.matmul(
                                    prec[cb][ch][:],
                                    bgT8[q][:, :, 128 * (9 * cb + d):
                                            128 * (9 * cb + d + 1)],
                                    T8p[q][:, :, z0:z0 + 16, x0:x0 + 32],
                                    start=(q == 0 and d == 0),
                                    stop=(q == 3 and d == 8), perf_mode=DR)

                for t in range(8):
                    emit_T8(t)
                for q in range(4):
                    emit_tconv(q)

                # pre = eps*boxs2*mask/9 + fg*(1-mask): ready during tconv
                final_pad = [main.tile([128, 48, 48], BF16,
                                       name=f"final_pad{c}") for c in range(2)]
                pre = [stage.tile([128, 32, 32], F32, name=f"pre{c}")
                       for c in range(2)]
                for c in range(2):
                    nc.gpsimd.tensor_tensor(pre[c][:], fflat[:, c],
                                            invmaskb[:], ALU.mult)
                    nc.gpsimd.tensor_tensor(pre[c][:], pre[c][:], s2b[:],
                                            ALU.add)
                    nc.gpsimd.memset(final_pad[c][:, 0:8, :].bitcast(U16), 0)
                    nc.gpsimd.memset(final_pad[c][:, 40:48, :].bitcast(U16), 0)
                    nc.gpsimd.memset(
                        final_pad[c][:, 8:40, 0:8].bitcast(U16), 0)
                    nc.gpsimd.memset(
                        final_pad[c][:, 8:40, 40:48].bitcast(U16), 0)

                fscr = [stage.tile([128, 32, 32], F32, name=f"fscr{i}")
                        for i in range(2)]
                # assembly per (cb, ch) half; fp8 hi/lo quantize split at
                # row 28 so the r<=4 ch0 conv groups start on chunk a
                for cb in range(2):
                    for ch in range(2):
                        r0 = 16 * ch
                        nc.vector.tensor_tensor(
                            fscr[cb][:, r0:r0 + 16, :], prec[cb][ch][:],
                            maskb9s[:, r0:r0 + 16, :], ALU.mult)
                        nc.vector.tensor_tensor(
                            final_pad[cb][:, 8 + r0:24 + r0, 8:40],
                            fscr[cb][:, r0:r0 + 16, :],
                            pre[cb][:, r0:r0 + 16, :], ALU.add)
                for cb in range(2):
                    nc.scalar.copy(f8h[:, cb, 0:28, :],
                                   final_pad[cb][:, 0:28, :])
                for cb in range(2):
                    nc.vector.scalar_tensor_tensor(
                        out=f8l[:, cb, 0:28, :],
                        in0=f8h[:, cb, 0:28, :], scalar=-1.0,
                        in1=final_pad[cb][:, 0:28, :],
                        op0=ALU.mult, op1=ALU.add)
                for cb in range(2):
                    nc.scalar.copy(f8h[:, cb, 28:48, :],
                                   final_pad[cb][:, 28:48, :])
                for cb in range(2):
                    nc.vector.scalar_tensor_tensor(
                        out=f8l[:, cb, 28:48, :],
                        in0=f8h[:, cb, 28:48, :], scalar=-1.0,
                        in1=final_pad[cb][:, 28:48, :],
                        op0=ALU.mult, op1=ALU.add)
            ps_db_cm.__exit__(None, None, None)
            ps_acc_cm.__exit__(None, None, None)
            stage_cm.__exit__(None, None, None)
        # ---------- work pool closed ----------

        with tc.tile_pool(name="late", bufs=1) as late:
            # ---------- dilated convs (fp8 DR, 3-term compensated) ----------
            out_sb = [late.tile([64, 32, 32], F32, name=f"out_sb{g}")
                      for g in range(4)]

            with tc.tile_pool(name="ps_o", bufs=4, space="PSUM") as pso_pool:
                for g, ch in ((0, 0), (1, 0), (2, 0), (0, 1), (1, 1),
                              (2, 1), (3, 0), (3, 1)):
                    if True:
                        r = RATES[g]
                        pso = pso_pool.tile([64, 512], F32, name="pso",
                                            tag="pso")
                        i = 0
                        for wt, ft in ((w8h, f8h), (w8h, f8l), (w8l, f8h)):
                            for d, (dy, dx) in enumerate(OFFS):
                                oy = 8 + r * (dy - 1) + 16 * ch
                                ox = 8 + r * (dx - 1)
                                woff = 576 * g + 64 * (3 * dy + dx)
                                nc.tensor.matmul(
                                    pso[:],
                                    wt[:, :, woff:woff + 64],
                                    ft[:, :, oy:oy + 16, ox:ox + 32],
                                    start=(i == 0), stop=(i == 26),
                                    perf_mode=DR)
                                i += 1
                        # split the last chunk so its DMA overlaps the relu
                        nsub = 2 if (g == 3 and ch == 1) else 1
                        for s in range(nsub):
                            rs = 16 * ch + (16 // nsub) * s
                            rn_ = 16 // nsub
                            nc.scalar.activation(
                                out_sb[g][:, rs:rs + rn_, :],
                                pso[:].rearrange("p (a b) -> p a b", b=32)
                                [:, rs - 16 * ch:rs - 16 * ch + rn_, :],
                                AF.Relu, bias=biasb[:, g:g + 1],
                                scale=1.0 / 32.0)
                            nc.sync.dma_start(
                                out_d[64 * g:64 * (g + 1), rs:rs + rn_, :],
                                out_sb[g][:, rs:rs + rn_, :])


def _get_nc():
    if "nc" not in _CACHE:
        _CACHE["nc"] = build_program()
    return _CACHE["nc"]


def kernel(foreground, mask, background, conv_w, conv_b):
    nc = _get_nc()
    fg = np.ascontiguousarray(foreground, dtype=np.float32).reshape(
        8, 2, 128, 32, 32).astype(ml_dtypes.bfloat16)

    def xvariants(x):
        # [8, 2, 128, 32, 32] -> [8, 128, 3, 2, 32, 32], v holds x+v-1
        z = np.zeros_like(x[..., :1])
        v = np.stack([
            np.concatenate([z, x[..., :-1]], axis=-1),
            x,
            np.concatenate([x[..., 1:], z], axis=-1),
        ], axis=1)                                   # [8, 3, 2, 128, 32, 32]
        return np.ascontiguousarray(v.transpose(0, 3, 1, 2, 4, 5))

    fg8 = xvariants(fg.astype(ml_dtypes.float8_e4m3))
    bgm = (np.ascontiguousarray(background, dtype=np.float32).reshape(
        8, 2, 128, 32, 32) * (1.0 - mask.reshape(1, 1, 1, 32, 32))
    ).astype(ml_dtypes.bfloat16)
    bgmc = np.ascontiguousarray(bgm.transpose(0, 2, 1, 3, 4))
    bg8 = xvariants(bgm.astype(ml_dtypes.float8_e4m3))
    maskrow = np.ascontiguousarray(mask.reshape(1, 1024), dtype=np.float32)
    # conv_w [4,64,256,3,3] -> [k, cb, g*9*64] fp8 hi/lo pair, x32 gain
    w32 = np.ascontiguousarray(
        conv_w.astype(np.float32).transpose(2, 0, 3, 4, 1).reshape(2, 128, 2304)
    ).transpose(1, 0, 2) * 32.0
    w8h = np.ascontiguousarray(w32).astype(ml_dtypes.float8_e4m3)
    w8l = np.ascontiguousarray(
        w32 - w8h.astype(np.float32)).astype(ml_dtypes.float8_e4m3)
    bias = np.ascontiguousarray(conv_b.astype(np.float32).reshape(4, 64).T)
    in_maps = [
        {"fg": fg[i], "fg8": fg8[i], "bgm": bgmc[i], "bg8": bg8[i],
         "maskrow": maskrow, "w8h": w8h, "w8l": w8l, "bias": bias}
        for i in range(8)
    ]
    res = run_bass_kernel_spmd(nc, in_maps, list(range(8)))
    return np.stack([res.results[i]["out"] for i in range(8)], axis=0)


if __name__ == "__main__":
    build_program()
    print("build ok")


# revision 102
# speedup vs baseline: 1.0938x; 1.0683x over previous
"""Trainium2 Bass kernel for ContextualAttentionModule.

Data-parallel over batch: 8 samples -> 8 NeuronCores, one sample per core.
Per-core pipeline (C=256, H=W=32, L=1024 patches):
  scores  = <fg_patch(p), bg_patch(l)> via fp8e4 DoubleRow matmuls, hi-term
            only (the recovered/attention branch carries ~3% of output
            energy, so fp8 rounding there is far under the error budget)
  norm    = rsqrt(3x3-boxsum(colsum bg_masked^2) + 2304*eps^2), bf16 boxsum
  prop    = 3x3 window-sum of scores + eps*boxbox(colsum fg)  (DVE, bf16)
  E       = exp(prop * rn) in bf16 (no max-subtract)
  D, s2   = interleaved per-block [1,512] PE accumulations over E (lag 2)
  T8      = fp8(E * (1/D) * min(rn*2048, 256))  -- the 2^11 gain keeps the
            attn operand in fp8's normal range; undone exactly via the
            mask/(9*2048) constant tile.  1/D read directly from PSUM.
  recov   = fp8 DoubleRow tconv: t-block pairs contract in one matmul
            (lhs = transposed masked-bg banks quantized to fp8, rhs = T8)
  final   = (recov*mask/(9*2048)*2048 + eps*box(s2/D))*... + fg*(1-mask)
  out     = concat_g relu(dilated_conv_r(final) + b)  bf16 weights
"""

import numpy as np
import ml_dtypes

import concourse.bass as bass
import concourse.tile as tile
from concourse import bacc, mybir
from concourse.bass_utils import run_bass_kernel_spmd
from concourse.masks import make_identity

F32 = mybir.dt.float32
F32R = mybir.dt.float32r
BF16 = mybir.dt.bfloat16
F8 = mybir.dt.float8e4
U8 = mybir.dt.uint8
U16 = mybir.dt.uint16
AF = mybir.ActivationFunctionType
ALU = mybir.AluOpType
DR = mybir.MatmulPerfMode.DoubleRow

EPS = 1e-7
TS = 2048.0          # power-of-2 gain on the attn operand for fp8 range
RATES = (1, 2, 4, 8)
OFFS = [(dy, dx) for dy in range(3) for dx in range(3)]
# dy=1 taps first so the start=True matmul covers the full psum tile
TAPS = [(1, 0), (1, 1), (1, 2), (0, 0), (0, 1), (0, 2), (2, 0), (2, 1), (2, 2)]

_CACHE = {}


def _ring_zero(nc, buf, n=34, eng=None):
    """Zero only the 1-wide border ring of a [P, n, n] padded buffer."""
    eng = eng or nc.vector
    bc = U16 if buf.dtype == BF16 else F32
    eng.memset(buf[:, 0:n:n - 1, :].bitcast(bc), 0)
    eng.memset(buf[:, 1:n - 1, 0:n:n - 1].bitcast(bc), 0)


def _ring_zero_pair(nc, buf, eng=None):
    """Ring-zero both j-halves of a [P, 2, 34, 34] paired buffer."""
    eng = eng or nc.vector
    bc = U16 if buf.dtype == BF16 else U8
    eng.memset(buf[:, :, 0:34:33, :].bitcast(bc), 0)
    eng.memset(buf[:, :, 1:33, 0:34:33].bitcast(bc), 0)


def _boxsum(nc, scr, src_pad, dst_flat, eng=None):
    """3x3 SAME window sum: [1,34,34] ring-zero padded -> [1,32,32] flat."""
    eng = eng or nc.vector
    eng.tensor_tensor(scr[:, 1:33, 1:33], src_pad[:, 1:33, 0:32],
                      src_pad[:, 1:33, 1:33], ALU.add)
    eng.tensor_tensor(scr[:, 1:33, 1:33], scr[:, 1:33, 1:33],
                      src_pad[:, 1:33, 2:34], ALU.add)
    eng.tensor_tensor(dst_flat[:], scr[:, 0:32, 1:33],
                      scr[:, 1:33, 1:33], ALU.add)
    eng.tensor_tensor(dst_flat[:], dst_flat[:], scr[:, 2:34, 1:33], ALU.add)


def build_program():
    nc = bacc.Bacc()
    fg_d = nc.declare_dram_parameter("fg", [2, 128, 32, 32], BF16, isOutput=False)
    fg8_d = nc.declare_dram_parameter("fg8", [128, 3, 2, 32, 32], F8,
                                      isOutput=False)
    bgb_d = nc.declare_dram_parameter("bgm", [128, 2, 32, 32], BF16,
                                      isOutput=False)
    bg8_d = nc.declare_dram_parameter("bg8", [128, 3, 2, 32, 32], F8,
                                      isOutput=False)
    mask_d = nc.declare_dram_parameter("maskrow", [1, 1024], F32, isOutput=False)
    w_d = nc.declare_dram_parameter("wconv", [2, 128, 1152], BF16,
                                    isOutput=False)
    w8h_d = nc.declare_dram_parameter("w8h", [128, 2, 2304], F8, isOutput=False)
    w8l_d = nc.declare_dram_parameter("w8l", [128, 2, 2304], F8, isOutput=False)
    b_d = nc.declare_dram_parameter("bias", [64, 4], F32, isOutput=False)
    out_d = nc.declare_dram_parameter("out", [256, 32, 32], F32, isOutput=True)

    with tile.TileContext(nc) as tc:
        _emit(nc, tc, fg_d, fg8_d, bgb_d, bg8_d, mask_d, w_d, w8h_d,
              w8l_d, b_d, out_d)
    nc.compile()
    return nc


def _emit(nc, tc, fg_d, fg8_d, bgb_d, bg8_d, mask_d, w_d, w8h_d,
          w8l_d, b_d, out_d):
    with tc.tile_pool(name="main", bufs=1) as main:
        # ---------------- long-lived tiles ----------------
        fflat = main.tile([128, 2, 32, 32], BF16, name="fflat")
        invmaskb = main.tile([128, 32, 32], F32, name="invmaskb")
        maskb9 = main.tile([128, 32, 32], F32, name="maskb9")
        maskb9s = main.tile([128, 32, 32], F32, name="maskb9s")
        ones_col = main.tile([128, 1], F32R, name="ones_col")
        idR = main.tile([128, 128], BF16, name="idR")
        rncol = main.tile([128, 8], F32, name="rncol")
        rnccol = main.tile([128, 8], F32, name="rnccol")
        ercol = main.tile([128, 8], BF16, name="ercol")
        onesB = main.tile([128, 1], BF16, name="onesB")
        Gb = main.tile([128, 32, 32], BF16, name="Gb")
        msrow = main.tile([1, 1024], F32, name="msrow")
        wsb = [main.tile([128, 1152], BF16, name=f"wsb{c}") for c in range(2)]
        w8h = main.tile([128, 2, 2304], F8, name="w8h")
        w8l = main.tile([128, 2, 2304], F8, name="w8l")
        f8h = main.tile([128, 2, 48, 48], F8, name="f8h")
        f8l = main.tile([128, 2, 48, 48], F8, name="f8l")
        biasb = main.tile([64, 4], F32, name="biasb")
        Ap = [main.tile([128, 2, 34, 34], BF16, name=f"Ap{q}") for q in range(4)]
        A = [Ap[t // 2][:, t % 2] for t in range(8)]
        T8p = [main.tile([128, 2, 34, 34], F8, name=f"T8p{q}") for q in range(4)]
        W = [main.tile([128, 34, 34], BF16, name=f"W{i}") for i in range(2)]

        with tc.tile_pool(name="work", bufs=1) as work:
            # masked-bg x-shift variants (host-prepared): index v holds
            # x[w + v - 1] (0 at edges); rows padded (34) for dy offsets.
            bgbT = work.tile([128, 3, 2, 34, 32], BF16, name="bgbT")
            bg8T = work.tile([128, 3, 2, 34, 32], F8, name="bg8T")
            fg8T = work.tile([128, 3, 2, 32, 32], F8, name="fg8T")
            bgb = [bgbT[:, v] for v in range(3)]
            bg8h = [bg8T[:, v] for v in range(3)]
            fg8h = [fg8T[:, v] for v in range(3)]
            bgT8 = [work.tile([128, 2, 2304], F8, name=f"bgT8{q}")
                    for q in range(4)]
            rdrow = work.tile([1, 1024], F32, name="rdrow")
            s2row = work.tile([1, 1024], F32, name="s2row")

            stage_cm = tc.tile_pool(name="stage", bufs=1)
            stage = stage_cm.__enter__()
            ps_acc_cm = tc.tile_pool(name="ps_acc", bufs=1, space="PSUM")
            ps_acc = ps_acc_cm.__enter__()
            ps_p1_cm = tc.tile_pool(name="ps_p1", bufs=1, space="PSUM")
            ps_p1 = ps_p1_cm.__enter__()
            ps_sc_cm = tc.tile_pool(name="ps_sc", bufs=3, space="PSUM")
            ps_sc = ps_sc_cm.__enter__()
            ps_tra_cm = tc.tile_pool(name="ps_tra", bufs=2, space="PSUM")
            ps_tra = ps_tra_cm.__enter__()

            q2 = stage.tile([128, 2, 32, 32], F32R, name="q2")
            onesf0 = stage.tile([128, 1], F32, name="onesf0")

            # ---------- phase 0: loads + edge zeros ----------
            nc.sync.dma_start(msrow[:], mask_d[:])
            nc.sync.dma_start(bg8T[:, :, :, 1:33, :], bg8_d[:])
            nc.sync.dma_start(fg8T[:], fg8_d[:])
            nc.sync.dma_start(bgbT[:, 1, :, 1:33, :], bgb_d[:])
            nc.sync.dma_start(fflat[:, 0], fg_d[0])
            nc.sync.dma_start(fflat[:, 1], fg_d[1])
            nc.sync.dma_start(wsb[0][:], w_d[0])
            nc.sync.dma_start(wsb[1][:], w_d[1])
            nc.sync.dma_start(w8h[:], w8h_d[:])
            nc.sync.dma_start(w8l[:], w8l_d[:])
            nc.sync.dma_start(biasb[:], b_d[:])

            nc.gpsimd.memset(onesf0[:], 1.0)
            # preload the sqrt table (phase1's only activation); the exp
            # table loads once right after the sqrt, under the scores.
            nc.scalar.activation(onesf0[0:1], onesf0[0:1], AF.Sqrt)
            nc.gpsimd.memset(onesf0[:], 1.0)

            # mask broadcast chain first in the DVE queue (head critical path)
            onesrowR = stage.tile([1, 128], F32R, name="onesrowR")
            nc.vector.memset(onesrowR[:].bitcast(F32), 1.0)
            msrowR = stage.tile([1, 1024], F32R, name="msrowR")
            nc.vector.tensor_copy(msrowR[:], msrow[:])
            idf = stage.tile([128, 128], F32, name="idf")
            make_identity(nc, idf[:])
            nc.gpsimd.tensor_copy(idR[:], idf[:])
            nc.gpsimd.tensor_copy(onesB[:], onesf0[:])
            nc.gpsimd.tensor_copy(ones_col[:], onesf0[:])
            for q in range(4):
                _ring_zero_pair(nc, Ap[q], eng=nc.gpsimd)
                _ring_zero_pair(nc, T8p[q], eng=nc.gpsimd)
            for w in W:
                _ring_zero(nc, w, eng=nc.gpsimd)
            nc.gpsimd.memset(bgbT[:, :, :, 0:34:33, :].bitcast(U16), 0)
            nc.gpsimd.memset(bg8T[:, :, :, 0:34:33, :].bitcast(U8), 0)
            nc.gpsimd.memset(bgbT[:, 0, :, 1:33, 0:1].bitcast(U16), 0)
            nc.gpsimd.memset(bgbT[:, 2, :, 1:33, 31:32].bitcast(U16), 0)
            # bgb x-variants from the loaded center (4x DVE copies)
            nc.vector.tensor_copy(bgbT[:, 0, :, 1:33, 1:32],
                                  bgbT[:, 1, :, 1:33, 0:31])
            nc.vector.tensor_copy(bgbT[:, 2, :, 1:33, 0:31],
                                  bgbT[:, 1, :, 1:33, 1:32])

            invmaskB = stage.tile([128, 32, 32], BF16, name="invmaskB")
            for h in range(2):
                psb = ps_p1.tile([128, 16, 32], F32, name="psb", tag="p1")
                nc.tensor.matmul(
                    psb[:].rearrange("p a b -> p (a b)"), onesrowR[:],
                    msrowR[:, 512 * h:512 * (h + 1)], start=True, stop=True)
                nc.vector.tensor_scalar(
                    out=invmaskB[:, 16 * h:16 * (h + 1), :], in0=psb[:],
                    scalar1=-1.0, scalar2=1.0, op0=ALU.mult, op1=ALU.add)

            # q2 = (masked bg)^2, feeding the patch-norm row sums
            nc.scalar.activation(q2[:, 0], bgb[1][:, 0, 1:33, :], AF.Square)
            nc.scalar.activation(q2[:, 1], bgb[1][:, 1, 1:33, :], AF.Square)

            # ---------- phase 1: patch norms + Gb (emitted before scores
            # so its PE matmuls run while the loads finish) ----
            rpad = stage.tile([1, 34, 34], BF16, name="rpad")
            rscr = stage.tile([1, 34, 34], BF16, name="rscr")
            _ring_zero(nc, rpad, eng=nc.gpsimd)
            _ring_zero(nc, rscr, eng=nc.gpsimd)
            sqrow = stage.tile([1, 1024], F32, name="sqrow")
            for ch in range(2):
                pr = ps_p1.tile([1, 512], F32, name="pr", tag="p1")
                for cb in range(2):
                    nc.tensor.matmul(
                        pr[:], ones_col[:],
                        q2[:, cb, 16 * ch:16 * ch + 16, :],
                        start=(cb == 0), stop=(cb == 1))
                nc.scalar.copy(rpad[:, 1 + 16 * ch:17 + 16 * ch, 1:33], pr[:])
            ssqrow = stage.tile([1, 32, 32], BF16, name="ssqrow")
            _boxsum(nc, rscr, rpad, ssqrow)
            urow = stage.tile([1, 1024], F32, name="urow")
            nc.vector.tensor_scalar_add(
                urow[:], ssqrow[:].rearrange("o a b -> o (a b)"),
                2304.0 * EPS * EPS)
            nc.scalar.activation(urow[:], urow[:], AF.Sqrt)
            # single switch to the exp table, hidden under early scores
            nc.scalar.activation(onesf0[0:1], onesf0[0:1], AF.Exp)
            nc.vector.reciprocal(urow[:], urow[:])
            # rncol[p, t] = rnrow[128t + p]: broadcast + diag extract
            Bb = stage.tile([128, 1024], F32, name="Bb")
            nc.gpsimd.partition_broadcast(Bb[:], urow[:])
            junk = stage.tile([128, 128], F32, name="junk")
            for t in range(8):
                nc.vector.scalar_tensor_tensor(
                    out=junk[:], in0=Bb[:, 128 * t:128 * (t + 1)],
                    scalar=1.0, in1=idf[:], op0=ALU.mult, op1=ALU.mult,
                    accum_out=rncol[:, t:t + 1])
            nc.vector.tensor_scalar(out=rnccol[:], in0=rncol[:], scalar1=TS,
                                    scalar2=256.0, op0=ALU.mult, op1=ALU.min)
            nc.vector.tensor_scalar_mul(ercol[:], rncol[:], EPS)

            # boxg: G = 3x3 box of channel-sum(fg); Gb = eps*box(G) bcast
            for ch in range(2):
                prg = ps_p1.tile([1, 512], F32, name="prg", tag="p1")
                for cb in range(2):
                    nc.tensor.matmul(
                        prg[:], onesB[:],
                        fflat[:, cb, 16 * ch:16 * ch + 16, :],
                        start=(cb == 0), stop=(cb == 1))
                nc.scalar.copy(rpad[:, 1 + 16 * ch:17 + 16 * ch, 1:33], prg[:])
            _boxsum(nc, rscr, rpad, ssqrow)
            nc.vector.tensor_copy(rpad[:, 1:33, 1:33], ssqrow[:])
            bbrow = stage.tile([1, 32, 32], BF16, name="bbrow")
            _boxsum(nc, rscr, rpad, bbrow)
            ebrowB = stage.tile([1, 1024], BF16, name="ebrowB")
            nc.vector.tensor_scalar_mul(
                ebrowB[:], bbrow[:].rearrange("o a b -> o (a b)"), EPS)
            nc.gpsimd.partition_broadcast(
                Gb.rearrange("p a b -> p (a b)"), ebrowB[:])

            # ---------- scores loop (fp8 DR, hi only) ----------
            # row-accumulators pair up in PSUM banks via partition offsets
            accD = ps_acc.tile([128, 512], F32, name="accD")
            accS = ps_acc.tile([128, 512], F32, name="accS")
            dAcc = [accD[64 * c:64 * c + 1, :] for c in range(2)]
            s2Acc = [accS[64 * c:64 * c + 1, :] for c in range(2)]

            def emit_scores(t, ch):
                psc = ps_sc.tile([128, 16, 32], F32, name="psc", tag="psc")
                for i, (dy, dx) in enumerate(TAPS):
                    fr = 16 * ch + dy - 1
                    r0, r1 = max(0, -fr), min(16, 32 - fr)
                    nc.tensor.matmul(
                        psc[:, r0:r1, :],
                        bg8h[dx][:, :, 4 * t + dy:4 * t + dy + 4, :],
                        fg8h[dx][:, :, fr + r0:fr + r1, :],
                        start=(i == 0), stop=(i == 8), perf_mode=DR)
                half = A[t][:, 1 + 16 * ch:17 + 16 * ch, 1:33]
                nc.scalar.copy(half, psc[:])

            hp_cm = tc.tile_pool(name="hp", bufs=2)
            hp = hp_cm.__enter__()

            def emit_boxsum_exp(t):
                w = W[t % 2]
                nc.vector.tensor_tensor(
                    w[:, 1:33, 1:33], A[t][:, 1:33, 0:32],
                    A[t][:, 1:33, 1:33], ALU.add)
                nc.vector.tensor_tensor(
                    w[:, 1:33, 1:33], w[:, 1:33, 1:33],
                    A[t][:, 1:33, 2:34], ALU.add)
                H = hp.tile([128, 32, 32], BF16, name="H", tag="H")
                nc.vector.tensor_tensor(
                    H[:], w[:, 0:32, 1:33], w[:, 1:33, 1:33], ALU.add)
                nc.vector.tensor_tensor(H[:], H[:], w[:, 2:34, 1:33], ALU.add)
                eng = nc.vector if t >= 6 else nc.gpsimd
                eng.tensor_tensor(H[:], H[:], Gb[:], ALU.add)
                for ch in range(2):
                    nc.scalar.activation(
                        A[t][:, 1 + 16 * ch:17 + 16 * ch, 1:33],
                        H[:, 16 * ch:16 * ch + 16, :],
                        AF.Exp, scale=rncol[:, t:t + 1])

            def emit_d_s2(t):
                for ch in range(2):
                    av = A[t][:, 1 + 16 * ch:17 + 16 * ch, 1:33]
                    nc.tensor.matmul(dAcc[ch][:], onesB[:], av,
                                     start=(t == 0), stop=(t == 7))
                    nc.tensor.matmul(s2Acc[ch][:], ercol[:, t:t + 1],
                                     av, start=(t == 0), stop=(t == 7))

            def build_bgT8(q):
                # transpose (j, cb, d) 128x128 blocks of shifted masked bg
                # into bf16 psum, 4 at a time, then fp8-quantize to SBUF.
                blocks = [(j, cb, d) for j in range(2) for cb in range(2)
                          for d in range(9)]
                for g in range(9):
                    chunk = blocks[4 * g:4 * g + 4]
                    ptr = ps_tra.tile([128, 512], BF16, name="ptr", tag="ptr")
                    for bi, (j, cb, d) in enumerate(chunk):
                        dy, dx = OFFS[d]
                        t = 2 * q + j
                        nc.tensor.transpose(
                            ptr[:, 128 * bi:128 * (bi + 1)],
                            bgb[dx][:, cb, 4 * t + dy:4 * t + dy + 4, :]
                            .rearrange("p a b -> p (a b)"),
                            idR[:])
                    dst = bgT8[q].rearrange("p j c -> p (j c)")[
                        :, 512 * g:512 * g + 512]
                    if g % 3 < 2:
                        nc.scalar.copy(dst, ptr[:])
                    else:
                        nc.vector.tensor_copy(dst, ptr[:])

            for t in range(8):
                emit_scores(t, 0)
                if t >= 3:
                    emit_d_s2(t - 3)
                emit_scores(t, 1)
                if t in (1, 2, 4, 5):
                    build_bgT8({1: 0, 2: 1, 4: 2, 5: 3}[t])
                emit_boxsum_exp(t)
            emit_d_s2(5)
            emit_d_s2(6)
            emit_d_s2(7)
            hp_cm.__exit__(None, None, None)
            ps_tra_cm.__exit__(None, None, None)
            ps_sc_cm.__exit__(None, None, None)
            ps_p1_cm.__exit__(None, None, None)

            # ---------- softmax denom + T8 + tconv ----------
            ps_db_cm = tc.tile_pool(name="ps_db", bufs=1, space="PSUM")
            ps_db = ps_db_cm.__enter__()
            psd = [ps_db.tile([128, 512], F32, name=f"psd{c}") for c in range(2)]
            rdrowR = stage.tile([1, 1024], F32R, name="rdrowR")
            Db = stage.tile([128, 32, 32], BF16, name="Db")
            for ch in range(2):
                nc.vector.reciprocal(
                    rdrow[:, 512 * ch:512 * (ch + 1)], dAcc[ch][:])
                nc.vector.tensor_copy(rdrowR[:, 512 * ch:512 * (ch + 1)],
                                      rdrow[:, 512 * ch:512 * (ch + 1)])
                nc.tensor.matmul(psd[ch][:], onesrowR[:],
                                 rdrowR[:, 512 * ch:512 * (ch + 1)],
                                 start=True, stop=True)
                nc.scalar.copy(
                    Db[:, 16 * ch:16 * ch + 16, :],
                    psd[ch][:].rearrange("p (a b) -> p a b", b=32))
            for ch in range(2):
                nc.vector.tensor_tensor(
                    s2row[:, 512 * ch:512 * (ch + 1)], s2Acc[ch][:],
                    rdrow[:, 512 * ch:512 * (ch + 1)], ALU.mult)

            def emit_T8(t):
                q, j = t // 2, t % 2
                if t < 2:
                    for ch in range(2):
                        nc.vector.scalar_tensor_tensor(
                            out=T8p[q][:, j, 1 + 16 * ch:17 + 16 * ch, 1:33],
                            in0=A[t][:, 1 + 16 * ch:17 + 16 * ch, 1:33],
                            scalar=rnccol[:, t:t + 1],
                            in1=Db[:, 16 * ch:16 * ch + 16, :],
                            op0=ALU.mult, op1=ALU.mult)
                else:
                    nc.vector.scalar_tensor_tensor(
                        out=T8p[q][:, j, 1:33, 1:33],
                        in0=A[t][:, 1:33, 1:33],
                        scalar=rnccol[:, t:t + 1],
                        in1=Db[:],
                        op0=ALU.mult, op1=ALU.mult)

            # mask constants (needed by the final assembly only)
            nc.gpsimd.tensor_copy(invmaskb[:], invmaskB[:])
            nc.gpsimd.tensor_scalar(
                out=maskb9[:], in0=invmaskB[:], scalar1=-1.0 / 9.0,
                scalar2=1.0 / 9.0, op0=ALU.mult, op1=ALU.add)
            nc.gpsimd.tensor_scalar(
                out=maskb9s[:], in0=invmaskB[:], scalar1=-1.0 / (9.0 * TS),
                scalar2=1.0 / (9.0 * TS), op0=ALU.mult, op1=ALU.add)

            # s2 path early: runs on Pool while T8/tconv proceed
            s2pad = W[0][0:1]
            s2scr = W[1][0:1]
            _ring_zero(nc, s2pad, eng=nc.gpsimd)
            _ring_zero(nc, s2scr, eng=nc.gpsimd)
            nc.gpsimd.tensor_copy(
                s2pad[:, 1:33, 1:33],
                s2row[:].rearrange("o (a b) -> o a b", b=32))
            boxs2 = stage.tile([1, 32, 32], BF16, name="boxs2")
            _boxsum(nc, s2scr, s2pad, boxs2, eng=nc.gpsimd)
            nc.gpsimd.tensor_scalar_mul(
                s2row[:], boxs2[:].rearrange("o a b -> o (a b)"), EPS)
            s2b = stage.tile([128, 32, 32], F32, name="s2b")
            nc.gpsimd.partition_broadcast(
                s2b.rearrange("p a b -> p (a b)"), s2row[:])
            nc.gpsimd.tensor_tensor(s2b[:], s2b[:], maskb9[:], ALU.mult)

            with tc.tile_pool(name="ps_rec", bufs=1, space="PSUM") as psrec:
                prec = [[psrec.tile([128, 512], F32, name=f"prec{c}_{ch}")
                         for ch in range(2)] for c in range(2)]

                def emit_tconv(q):
                    for cb in range(2):
                        for ch in range(2):
                            for d, (dy, dx) in enumerate(OFFS):
                                z0 = 16 * ch + 2 - dy
                                x0 = 2 - dx
                                nc.tensor.matmul(
                                    prec[cb][ch][:],
                                    bgT8[q][:, :, 128 * (9 * cb + d):
                                            128 * (9 * cb + d + 1)],
                                    T8p[q][:, :, z0:z0 + 16, x0:x0 + 32],
                                    start=(q == 0 and d == 0),
                                    stop=(q == 3 and d == 8), perf_mode=DR)

                for t in range(8):
                    emit_T8(t)
                for q in range(4):
                    emit_tconv(q)

                # pre = eps*boxs2*mask/9 + fg*(1-mask): ready during tconv
                final_pad = [main.tile([128, 48, 48], BF16,
                                       name=f"final_pad{c}") for c in range(2)]
                pre = [stage.tile([128, 32, 32], F32, name=f"pre{c}")
                       for c in range(2)]
                for c in range(2):
                    nc.gpsimd.tensor_tensor(pre[c][:], fflat[:, c],
                                            invmaskb[:], ALU.mult)
                    nc.gpsimd.tensor_tensor(pre[c][:], pre[c][:], s2b[:],
                                            ALU.add)
                    nc.gpsimd.memset(final_pad[c][:, 0:8, :].bitcast(U16), 0)
                    nc.gpsimd.memset(final_pad[c][:, 40:48, :].bitcast(U16), 0)
                    nc.gpsimd.memset(
                        final_pad[c][:, 8:40, 0:8].bitcast(U16), 0)
                    nc.gpsimd.memset(
                        final_pad[c][:, 8:40, 40:48].bitcast(U16), 0)

                fscr = [stage.tile([128, 32, 32], F32, name=f"fscr{i}")
                        for i in range(2)]
                # assembly per (cb, ch) half; fp8 hi/lo quantize split at
                # row 28 so the r<=4 ch0 conv groups start on chunk a
                for cb in range(2):
                    for ch in range(2):
                        r0 = 16 * ch
                        nc.vector.tensor_tensor(
                            fscr[cb][:, r0:r0 + 16, :], prec[cb][ch][:],
                            maskb9s[:, r0:r0 + 16, :], ALU.mult)
                        nc.vector.tensor_tensor(
                            final_pad[cb][:, 8 + r0:24 + r0, 8:40],
                            fscr[cb][:, r0:r0 + 16, :],
                            pre[cb][:, r0:r0 + 16, :], ALU.add)
                for cb in range(2):
                    nc.scalar.copy(f8h[:, cb, 0:28, :],
                                   final_pad[cb][:, 0:28, :])
                for cb in range(2):
                    nc.vector.scalar_tensor_tensor(
                        out=f8l[:, cb, 0:28, :],
                        in0=f8h[:, cb, 0:28, :], scalar=-1.0,
                        in1=final_pad[cb][:, 0:28, :],
                        op0=ALU.mult, op1=ALU.add)
                for cb in range(2):
                    nc.scalar.copy(f8h[:, cb, 28:48, :],
                                   final_pad[cb][:, 28:48, :])
                for cb in range(2):
                    nc.vector.scalar_tensor_tensor(
                        out=f8l[:, cb, 28:48, :],
                        in0=f8h[:, cb, 28:48, :], scalar=-1.0,
                        in1=final_pad[cb][:, 28:48, :],
                        op0=ALU.mult, op1=ALU.add)
            ps_db_cm.__exit__(None, None, None)
            ps_acc_cm.__exit__(None, None, None)
            stage_cm.__exit__(None, None, None)
        # ---------- work pool closed ----------

        with tc.tile_pool(name="late", bufs=1) as late:
            # ---------- dilated convs (fp8 DR, 3-term compensated) ----------
            out_sb = [late.tile([64, 32, 32], F32, name=f"out_sb{g}")
                      for g in range(4)]

            with tc.tile_pool(name="ps_o", bufs=4, space="PSUM") as pso_pool:
                for g, ch, fmt in ((0, 0, 'b'), (1, 0, 'b'), (0, 1, '8'),
                                   (1, 1, '8'), (2, 0, '8'), (2, 1, '8'),
                                   (3, 0, '8'), (3, 1, '8')):
                    if True:
                        r = RATES[g]
                        pso = pso_pool.tile([64, 512], F32, name="pso",
                                            tag="pso")
                        i = 0
                        if fmt == 'b':
                            # bf16 path: no quantize dependency; keeps PE
                            # busy while f8h/f8l are produced
                            for c in range(2):
                                for d, (dy, dx) in enumerate(OFFS):
                                    oy = 8 + r * (dy - 1) + 16 * ch
                                    ox = 8 + r * (dx - 1)
                                    woff = 576 * g + 64 * (3 * dy + dx)
                                    nc.tensor.matmul(
                                        pso[:],
                                        wsb[c][:, woff:woff + 64],
                                        final_pad[c][:, oy:oy + 16,
                                                     ox:ox + 32],
                                        start=(i == 0), stop=(i == 17))
                                    i += 1
                        else:
                            for wt, ft in ((w8h, f8h), (w8h, f8l),
                                           (w8l, f8h)):
                                for d, (dy, dx) in enumerate(OFFS):
                                    oy = 8 + r * (dy - 1) + 16 * ch
                                    ox = 8 + r * (dx - 1)
                                    woff = 576 * g + 64 * (3 * dy + dx)
                                    nc.tensor.matmul(
                                        pso[:],
                                        wt[:, :, woff:woff + 64],
                                        ft[:, :, oy:oy + 16, ox:ox + 32],
                                        start=(i == 0), stop=(i == 26),
                                        perf_mode=DR)
                                    i += 1
                        # split the last chunk so its DMA overlaps the relu
                        nsub = 2 if (g == 3 and ch == 1) else 1
                        for s in range(nsub):
                            rs = 16 * ch + (16 // nsub) * s
                            rn_ = 16 // nsub
                            nc.scalar.activation(
                                out_sb[g][:, rs:rs + rn_, :],
                                pso[:].rearrange("p (a b) -> p a b", b=32)
                                [:, rs - 16 * ch:rs - 16 * ch + rn_, :],
                                AF.Relu, bias=biasb[:, g:g + 1],
                                scale=(1.0 if fmt == 'b' else 1.0 / 32.0))
                            nc.sync.dma_start(
                                out_d[64 * g:64 * (g + 1), rs:rs + rn_, :],
                                out_sb[g][:, rs:rs + rn_, :])


def _get_nc():
    if "nc" not in _CACHE:
        _CACHE["nc"] = build_program()
    return _CACHE["nc"]


def kernel(foreground, mask, background, conv_w, conv_b):
    nc = _get_nc()
    fg = np.ascontiguousarray(foreground, dtype=np.float32).reshape(
        8, 2, 128, 32, 32).astype(ml_dtypes.bfloat16)

    def xvariants(x):
        # [8, 2, 128, 32, 32] -> [8, 128, 3, 2, 32, 32], v holds x+v-1
        z = np.zeros_like(x[..., :1])
        v = np.stack([
            np.concatenate([z, x[..., :-1]], axis=-1),
            x,
            np.concatenate([x[..., 1:], z], axis=-1),
        ], axis=1)                                   # [8, 3, 2, 128, 32, 32]
        return np.ascontiguousarray(v.transpose(0, 3, 1, 2, 4, 5))

    fg8 = xvariants(fg.astype(ml_dtypes.float8_e4m3))
    bgm = (np.ascontiguousarray(background, dtype=np.float32).reshape(
        8, 2, 128, 32, 32) * (1.0 - mask.reshape(1, 1, 1, 32, 32))
    ).astype(ml_dtypes.bfloat16)
    bgmc = np.ascontiguousarray(bgm.transpose(0, 2, 1, 3, 4))
    bg8 = xvariants(bgm.astype(ml_dtypes.float8_e4m3))
    maskrow = np.ascontiguousarray(mask.reshape(1, 1024), dtype=np.float32)
    # conv_w [4,64,256,3,3] -> [k, cb, g*9*64] fp8 hi/lo pair, x32 gain
    w32 = np.ascontiguousarray(
        conv_w.astype(np.float32).transpose(2, 0, 3, 4, 1).reshape(2, 128, 2304)
    ).transpose(1, 0, 2) * 32.0
    w8h = np.ascontiguousarray(w32).astype(ml_dtypes.float8_e4m3)
    w8l = np.ascontiguousarray(
        w32 - w8h.astype(np.float32)).astype(ml_dtypes.float8_e4m3)
    wreb = np.ascontiguousarray(
        w32.transpose(1, 0, 2)[:, :, 0:1152] / 32.0).astype(ml_dtypes.bfloat16)
    bias = np.ascontiguousarray(conv_b.astype(np.float32).reshape(4, 64).T)
    in_maps = [
        {"fg": fg[i], "fg8": fg8[i], "bgm": bgmc[i], "bg8": bg8[i],
         "maskrow": maskrow, "wconv": wreb, "w8h": w8h, "w8l": w8l,
         "bias": bias}
        for i in range(8)
    ]
    res = run_bass_kernel_spmd(nc, in_maps, list(range(8)))
    return np.stack([res.results[i]["out"] for i in range(8)], axis=0)


if __name__ == "__main__":
    build_program()
    print("build ok")
